# revision 1
# baseline (speedup 1.0000x reference)
"""Bilinear RoI pooling, V3: gather-free, SBUF-resident feature map.

Instead of DMA-gathering 4 KiB of fp16 texels per sample (103 MB/core of
HBM traffic), the whole fp16 feature map lives in SBUF, tiled as

    F4[p = yp*64 + xo, (g, B, c)] = feats[y = 2*g + yp, x = 63*B + xo, c]

(g = y-pair 0..31, B = x-block 0..4 of width 63, zero-padded past x=255).
Each sample (one output pixel of one RoI) becomes a 4-hot column in a
sparse fp16 rhs: its four bilinear weights sit at partition rows
(yp, xo), (yp, xo+1) for its two feature rows.  A matmul per
(y-pair g, band v, x-block B, channel chunk q) then computes

    psum[c, s] = sum_k w_k[s] * feats[y_k, x_k, c]

directly against the resident F4 slice -- the PE does the gather.

Samples are sorted by (y0, x0) and dealt round-robin to the 8 cores, so
per-(y0, x-block) segment quotas (max over cores) give ONE static graph
for all cores with ~1-2%% padding.  The graph is compiled per input
distribution (cached on the quota table).  Output is stored fp16 in
sorted-column order and un-permuted / upcast on the host.

Per-core DMA drops to ~57 MB (21 F4 + ~10 rhs + ~26 stores) vs ~155 MB
for the gather design."""

import hashlib

import numpy as np

HH, WW = 7, 7
C, Hf, Wf = 512, 64, 256
NPY, NPX = Hf - 1, Wf - 1         # base grids: y0 in 0..62, x0 in 0..254
N_CORES = 8
B_TOTAL = 4096
S_CORE = B_TOTAL * HH * WW // N_CORES   # 25088 samples per core
NG = 32                           # y-pairs
NB = 5                            # x-blocks of width 63
XBW = 63
SSUP = 256                        # psum super-block columns (2 banks; 4 supers in flight)
STBUF = 8
RBUF = 8                          # rhs slab buffers

_NC_CACHE = {}


def _build_nc(plan):
    import concourse.bacc as bacc
    import concourse.mybir as mybir

    seg = plan["seg"]            # (63, 5, 2) int: column [start, end) per (v, B)
    ncol = plan["ncol"]
    g_lo, g_hi = plan["g_lo"], plan["g_hi"]    # (32,) chain windows
    assert ncol % SSUP == 0
    n_supers = ncol // SSUP
    w_max = int(max(g_hi[g] - g_lo[g] for g in range(NG)))
    r_off = [0]
    for g in range(NG):
        r_off.append(r_off[-1] + (g_hi[g] - g_lo[g]))
    rhs_cols = r_off[-1]

    def bands(g):
        return [v for v in (2 * g - 1, 2 * g, 2 * g + 1) if 0 <= v <= 62]

    nc = bacc.Bacc("TRN2", debug=False)
    f16, f32 = mybir.dt.float16, mybir.dt.float32

    f4_d = nc.dram_tensor("f4", [128, NG * 4 * C], f16, kind="ExternalInput")
    f4b_d = nc.dram_tensor("f4b", [128, NG * C], f16, kind="ExternalInput")
    rhs_d = nc.dram_tensor("rhs", [128, rhs_cols], f16, kind="ExternalInput")
    out_d = nc.dram_tensor("out", [n_supers, 128, 4 * SSUP], f16, kind="ExternalOutput")

    f4 = nc.alloc_sbuf_tensor("f4_sb", [128, NG, 4 * C], f16)
    f4b = nc.alloc_sbuf_tensor("f4b_sb", [128, NG * C], f16)
    rb = [nc.alloc_sbuf_tensor(f"rb{i}", [128, w_max], f16) for i in range(RBUF)]
    st = [nc.alloc_sbuf_tensor(f"st{i}", [128, 4, SSUP], f16) for i in range(STBUF)]
    ps = nc.alloc_psum_tensor("ps", [128, 8 * 512], f32)

    # DMA completions are out-of-order: group F4 loads into phases of 2
    # slices (one sem each, threshold = both done; even slice on sync,
    # odd on scalar so the 21 MB load uses both queues) and give rhs
    # slabs per-slot sems (slot reuse is ordered through peg_sem).
    f_sems = [nc.alloc_semaphore(f"f_sem{i}") for i in range(NG // 2)]
    b4_sem = nc.alloc_semaphore("b4_sem")
    r_sems = [nc.alloc_semaphore(f"r_sem{i}") for i in range(RBUF)]
    pe_sem = nc.alloc_semaphore("pe_sem")      # supers fully accumulated
    peg_sem = nc.alloc_semaphore("peg_sem")    # chains retired (rhs buf reuse)
    cpv_sem = nc.alloc_semaphore("cpv_sem")    # DVE copies (all 4 chunks) per super
    st_sems = [nc.alloc_semaphore(f"st_sem{i}") for i in range(STBUF)]

    # ---- static matmul schedule with super first/last touch bookkeeping ----
    # v-major: each column range's accumulation group closes within 1-2
    # adjacent matmuls (even y0: one start&stop; odd y0: start on chain
    # g=v//2, stop immediately after on chain g+1 from the next y-pair).
    # instruction list: (g, v, B, q, c0, c1, sa, start, stop)
    sched = []
    for v in range(NPY):
        for B in range(NB):
            c0, c1 = int(seg[v, B, 0]), int(seg[v, B, 1])
            if c1 <= c0:
                continue
            for q in range(4):
                a = c0
                while a < c1:
                    sa = a // SSUP
                    b = min(c1, (sa + 1) * SSUP)
                    if v % 2 == 0:
                        sched.append((v // 2, v, B, q, a, b, sa, True, True))
                    else:
                        sched.append((v // 2, v, B, q, a, b, sa, True, False))
                        sched.append(((v + 1) // 2, v, B, q, a, b, sa, False, True))
                    a = b
    first_touch = {}
    last_touch = {}
    for i, ins in enumerate(sched):
        sa = ins[6]
        first_touch.setdefault(sa, i)
        last_touch[sa] = i
    assert set(first_touch) == set(range(n_supers)), "super coverage hole"
    # monotone last-touch so pe_sem increments in super order
    lt = [last_touch[s] for s in range(n_supers)]
    assert lt == sorted(lt), "non-monotone super retirement"
    inc_at = {i: s for s, i in last_touch.items()}
    wait_at = {i: s for s, i in first_touch.items()}
    # last instruction of each chain g (for rhs buffer reuse)
    g_last = {}
    for i, ins in enumerate(sched):
        g_last[ins[0]] = i
    g_inc_at = {i: g for g, i in g_last.items()}
    # first instruction of each chain g (for load waits)
    g_first = {}
    for i, ins in enumerate(sched):
        g_first.setdefault(ins[0], i)

    # supers whose copies must be done before PE reaches instruction i:
    # bank set (sa % 2) previously used by super sa - 2

    # scalar engine needs rhs slabs loaded ahead of PE: chain g is needed
    # once PE hits g_first[g]; the store for super s transitively requires
    # chains up to the one retiring s.  Emit load g before the store whose
    # super's last_touch instruction index >= g_first[g].
    def g_needed_by_super(s):
        i = last_touch[s]
        out = 0
        for g in range(NG):
            if g_first[g] <= i:
                out = g
        return out

    with nc.Block() as block:

        @block.sync
        def _(sync):
            sync.dma_start(f4b[:, :], f4b_d[:, :]).then_inc(b4_sem, 16)
            for g in range(0, NG, 2):
                sync.dma_start(
                    f4[:, g, :], f4_d[:, g * 4 * C : (g + 1) * 4 * C]
                ).then_inc(f_sems[g // 2], 16)
            tail0 = max(n_supers - 16, 0)
            for s in range(n_supers):
                if s >= tail0 and s % 2 == 1:
                    continue                      # scalar stores the tail odds
                sync.wait_ge(cpv_sem, s + 1)
                sync.dma_start(
                    out_d[s], st[s % STBUF][:, :, :].rearrange("p j r -> p (j r)")
                ).then_inc(st_sems[s % STBUF], 16)
            for i in range(min(STBUF, n_supers)):
                sync.wait_ge(st_sems[i], 16 * ((n_supers - 1 - i) // STBUF + 1))

        @block.scalar
        def _(scalar):
            emitted = [0]
            f_odd = [1]

            def load_f4_until(jmax):
                while f_odd[0] <= min(jmax, NG - 1):
                    j = f_odd[0]
                    scalar.dma_start(
                        f4[:, j, :], f4_d[:, j * 4 * C : (j + 1) * 4 * C]
                    ).then_inc(f_sems[j // 2], 16)
                    f_odd[0] += 2

            def load_until(gmax):
                while emitted[0] <= min(gmax, NG - 1):
                    g = emitted[0]
                    load_f4_until(2 * g + 3)
                    if g >= RBUF:
                        scalar.wait_ge(peg_sem, g - RBUF + 1)
                    wg = g_hi[g] - g_lo[g]
                    scalar.dma_start(
                        rb[g % RBUF][:, 0:wg],
                        rhs_d[:, r_off[g] : r_off[g] + wg],
                    ).then_inc(r_sems[g % RBUF], 16)
                    emitted[0] += 1
                load_f4_until(2 * emitted[0] + 3)

            load_until(NG - 1)
            load_f4_until(NG - 1)
            for s in range(max(n_supers - 16, 0), n_supers):
                if s % 2 == 1:
                    scalar.wait_ge(cpv_sem, s + 1)
                    scalar.dma_start(
                        out_d[s], st[s % STBUF][:, :, :].rearrange("p j r -> p (j r)")
                    ).then_inc(st_sems[s % STBUF], 16)

        @block.tensor
        def _(tensor):
            tensor.wait_ge(b4_sem, 16)
            seen_g = set()
            for i, (g, v, B, q, a, b, sa, st_, sp_) in enumerate(sched):
                if g not in seen_g:
                    seen_g.add(g)
                    tensor.wait_ge(f_sems[g // 2], 32)
                    tensor.wait_ge(r_sems[g % RBUF], 16 * (g // RBUF + 1))
                if i in wait_at:
                    s = wait_at[i]
                    if s >= 4:
                        tensor.wait_ge(cpv_sem, s - 3)
                off_ps = (sa % 4) * 1024 + q * SSUP
                o0, o1 = a - sa * SSUP, b - sa * SSUP
                lhsT = (
                    f4b[:, g * C + 128 * q : g * C + 128 * (q + 1)]
                    if B == 4
                    else f4[:, g, (B * C + 128 * q) : (B * C + 128 * (q + 1))]
                )
                mm = tensor.matmul(
                    ps[:, off_ps + o0 : off_ps + o1],
                    lhsT,
                    rb[g % RBUF][:, a - g_lo[g] : b - g_lo[g]],
                    start=st_,
                    stop=sp_,
                    skip_group_check=True,
                )
                if i in inc_at:
                    mm.then_inc(pe_sem, 1)
                    if i in g_inc_at:
                        tensor.nop().then_inc(peg_sem, 1)
                elif i in g_inc_at:
                    mm.then_inc(peg_sem, 1)

        @block.vector
        def _(vector):
            for s in range(n_supers):
                vector.wait_ge(pe_sem, s + 1)
                if s >= STBUF:
                    vector.wait_ge(st_sems[s % STBUF], 16 * (s // STBUF))
                off = (s % 4) * 4 * SSUP
                src_ap = ps[:, off : off + 4 * SSUP].rearrange(
                    "p (j r) -> p j r", r=SSUP
                )
                vector.tensor_copy(st[s % STBUF][:, :, :], src_ap).then_inc(
                    cpv_sem, 1
                )

    nc.compile()
    return nc


def _get_nc(plan):
    key = hashlib.sha256(
        plan["seg"].tobytes()
        + plan["g_lo"].tobytes()
        + plan["g_hi"].tobytes()
        + np.int64(plan["ncol"]).tobytes()
    ).hexdigest()
    if key not in _NC_CACHE:
        _NC_CACHE[key] = _build_nc(plan)
    return _NC_CACHE[key]


def _host_prep(feats, boxes, img_height, img_width):
    """Per-sample base row (y0*255 + x0, clamped) and 4 slot weights
    (tl, tr, bl, br with validity and clamp-aggregation folded in),
    mirroring the reference math in f32."""
    B = boxes.shape[0]
    f32 = np.float32
    xc, yc, w, h = (boxes[:, k].astype(f32) for k in range(4))
    tx = np.linspace(-1.0, 1.0, WW, dtype=f32)
    ty = np.linspace(-1.0, 1.0, HH, dtype=f32)
    inv_w = f32(1.0) / f32(img_width - 1)
    inv_h = f32(1.0) / f32(img_height - 1)
    gx = (f32(2.0) * xc[:, None] - f32(img_width - 1)) * inv_w \
        + (w * inv_w)[:, None] * tx[None, :]
    gy = (f32(2.0) * yc[:, None] - f32(img_height - 1)) * inv_h \
        + (h * inv_h)[:, None] * ty[None, :]
    px = (gx + f32(1.0)) * f32(0.5) * f32(Wf - 1)   # (B, WW)
    py = (gy + f32(1.0)) * f32(0.5) * f32(Hf - 1)   # (B, HH)

    x0 = np.floor(px)
    y0 = np.floor(py)
    fx, fy = px - x0, py - y0
    x0i, y0i = x0.astype(np.int64), y0.astype(np.int64)
    x1i, y1i = x0i + 1, y0i + 1
    vx0 = ((x0i >= 0) & (x0i <= Wf - 1)).astype(f32)
    vx1 = ((x1i >= 0) & (x1i <= Wf - 1)).astype(f32)
    vy0 = ((y0i >= 0) & (y0i <= Hf - 1)).astype(f32)
    vy1 = ((y1i >= 0) & (y1i <= Hf - 1)).astype(f32)
    x0c = np.clip(x0i, 0, Wf - 1).astype(np.int32)
    x1c = np.clip(x1i, 0, Wf - 1).astype(np.int32)
    y0c = np.clip(y0i, 0, Hf - 1).astype(np.int32)
    y1c = np.clip(y1i, 0, Hf - 1).astype(np.int32)

    def by(a):
        return np.broadcast_to(a[:, :, None], (B, HH, WW))

    def bx(a):
        return np.broadcast_to(a[:, None, :], (B, HH, WW))

    base_y = np.clip(y0i, 0, NPY - 1)                 # (B, HH)
    base_x = np.clip(x0i, 0, NPX - 1)                 # (B, WW)
    rows = (by(base_y) * NPX + bx(base_x)).reshape(-1).astype(np.int32)

    wx0, wx1 = f32(1.0) - fx, fx
    wy0, wy1 = f32(1.0) - fy, fy
    wk = np.stack(
        [
            by(wy0 * vy0) * bx(wx0 * vx0),
            by(wy0 * vy0) * bx(wx1 * vx1),
            by(wy1 * vy1) * bx(wx0 * vx0),
            by(wy1 * vy1) * bx(wx1 * vx1),
        ],
        axis=-1,
    ).reshape(B * HH * WW, 4).astype(f32)
    dy = np.stack(
        [by(y0c - base_y), by(y0c - base_y), by(y1c - base_y), by(y1c - base_y)],
        axis=-1,
    ).reshape(B * HH * WW, 4)
    dx = np.stack(
        [bx(x0c - base_x), bx(x1c - base_x), bx(x0c - base_x), bx(x1c - base_x)],
        axis=-1,
    ).reshape(B * HH * WW, 4)
    slots = np.clip(dy, 0, 1) * 2 + np.clip(dx, 0, 1)
    wts = np.zeros((B * HH * WW, 4), f32)
    np.add.at(wts, (np.arange(B * HH * WW)[:, None], slots), wk)
    return rows, wts


def _prepare(feats, boxes, img_height, img_width):
    rows, wts = _host_prep(feats, boxes, img_height, img_width)
    n = rows.shape[0]
    y0 = rows // NPX                   # 0..62
    x0 = rows % NPX                    # 0..254
    order = np.lexsort((x0, y0))
    percore = [order[m::N_CORES] for m in range(N_CORES)]   # (y0,x0)-sorted

    # per-(core, v, B) counts -> shared quotas
    cnt = np.zeros((N_CORES, NPY, NB), np.int64)
    for m in range(N_CORES):
        ids = percore[m]
        np.add.at(cnt[m], (y0[ids], x0[ids] // XBW), 1)
    qb = cnt.max(axis=0)               # (63, 5)
    ncol = int(qb.sum())
    pad = (-ncol) % SSUP
    qb[NPY - 1, NB - 1] += pad         # tail pad inside band 62 / block 4
    ncol += pad
    seg = np.zeros((NPY, NB, 2), np.int64)
    off = 0
    for v in range(NPY):
        for B in range(NB):
            seg[v, B] = (off, off + qb[v, B])
            off += qb[v, B]
    assert off == ncol
    bucket_lo = seg[:, 0, 0]
    bucket_hi = seg[:, NB - 1, 1]
    g_lo = np.array(
        [bucket_lo[max(2 * g - 1, 0)] for g in range(NG)], np.int64
    )
    g_hi = np.array(
        [bucket_hi[min(2 * g + 1, NPY - 1)] for g in range(NG)], np.int64
    )
    plan = {"seg": seg, "ncol": ncol, "g_lo": g_lo, "g_hi": g_hi}

    # F4 table (shared across cores)
    yp = np.arange(128) // 64          # (128,)
    xo = np.arange(128) % 64
    gs = np.arange(NG)
    Bs = np.arange(NB)
    yy = 2 * gs[None, :, None] + yp[:, None, None]          # (128, 32, 1)
    xx = XBW * Bs[None, None, :] + xo[:, None, None]        # (128, 1, 5)
    yy = np.broadcast_to(yy, (128, NG, NB))
    xx = np.broadcast_to(xx, (128, NG, NB))
    valid = xx < Wf
    xxc = np.minimum(xx, Wf - 1)
    ftab = feats.astype(np.float16)                          # (C, Hf, Wf)
    f4 = ftab[:, yy, xxc]                                    # (C, 128, 32, 5)
    f4 = f4 * valid[None].astype(np.float16)
    f4_full = f4.transpose(1, 2, 3, 0)                       # (128, 32, 5, C)
    f4_d = np.ascontiguousarray(f4_full[:, :, 0:4, :].reshape(128, NG * 4 * C))
    f4b_d = np.ascontiguousarray(f4_full[:, :, 4, :].reshape(128, NG * C))

    # per-core rhs slabs + column map
    r_off = np.zeros(NG + 1, np.int64)
    for g in range(NG):
        r_off[g + 1] = r_off[g] + (g_hi[g] - g_lo[g])
    rhs_cols = int(r_off[NG])

    in_maps = []
    colmaps = []
    for m in range(N_CORES):
        ids = percore[m]
        vv, bb = y0[ids], x0[ids] // XBW
        # column of each sample: seg start + rank within its (v, B) cell
        cell = vv * NB + bb
        o = np.argsort(cell, kind="stable")     # keeps x0-sorted order in cell
        ranks = np.empty(len(ids), np.int64)
        cc = cell[o]
        starts = np.r_[0, np.flatnonzero(cc[1:] != cc[:-1]) + 1]
        lens = np.diff(np.r_[starts, len(cc)])
        rr = np.concatenate([np.arange(L) for L in lens]) if len(cc) else cc
        ranks[o] = rr
        cols = seg[vv, bb, 0] + ranks
        colmap = np.full(ncol, -1, np.int64)
        colmap[cols] = ids
        colmaps.append(colmap)

        # dense rhs per chain g
        rhs = np.zeros((128, rhs_cols), np.float16)
        w4 = wts[ids]                            # (n, 4) tl,tr,bl,br
        xow = x0[ids] - XBW * bb                 # 0..62
        for g in range(NG):
            lo, hi = int(g_lo[g]), int(g_hi[g])
            sel = (cols >= lo) & (cols < hi)
            c_rel = cols[sel] - lo + r_off[g]
            v_s = vv[sel]
            xo_s = xow[sel]
            w_s = w4[sel]
            even = v_s == 2 * g
            high = v_s == 2 * g + 1
            low = v_s == 2 * g - 1
            # rows (yp, xo): row y0 -> yp = v - 2g; row y0+1 -> yp+1
            e_i = np.flatnonzero(even)
            if len(e_i):
                rhs[xo_s[e_i], c_rel[e_i]] += w_s[e_i, 0]
                rhs[xo_s[e_i] + 1, c_rel[e_i]] += w_s[e_i, 1]
                rhs[64 + xo_s[e_i], c_rel[e_i]] += w_s[e_i, 2]
                rhs[64 + xo_s[e_i] + 1, c_rel[e_i]] += w_s[e_i, 3]
            h_i = np.flatnonzero(high)
            if len(h_i):
                rhs[64 + xo_s[h_i], c_rel[h_i]] += w_s[h_i, 0]
                rhs[64 + xo_s[h_i] + 1, c_rel[h_i]] += w_s[h_i, 1]
            l_i = np.flatnonzero(low)
            if len(l_i):
                rhs[xo_s[l_i], c_rel[l_i]] += w_s[l_i, 2]
                rhs[xo_s[l_i] + 1, c_rel[l_i]] += w_s[l_i, 3]
        in_maps.append(
            {"f4": f4_d, "f4b": f4b_d, "rhs": np.ascontiguousarray(rhs)}
        )

    return plan, in_maps, colmaps


def kernel(**inputs):
    from concourse.bass_utils import run_bass_kernel_spmd

    feats = np.asarray(inputs["feats"], dtype=np.float32)
    boxes = np.asarray(inputs["boxes"], dtype=np.float32)
    img_height = int(np.asarray(inputs["img_height"]))
    img_width = int(np.asarray(inputs["img_width"]))

    plan, in_maps, colmaps = _prepare(feats, boxes, img_height, img_width)
    nc = _get_nc(plan)
    res = run_bass_kernel_spmd(nc, in_maps, core_ids=list(range(N_CORES)))

    out_all = np.empty((C, B_TOTAL * HH * WW), np.float32)
    for m, r in enumerate(res.results):
        a = r["out"]                                  # (S, 128, 4*392) f16
        S = a.shape[0]
        a = a.reshape(S, 128, 4, SSUP).transpose(2, 1, 0, 3).reshape(C, S * SSUP)
        cm = colmaps[m]
        valid = cm >= 0
        out_all[:, cm[valid]] = a[:, valid].astype(np.float32)
    out = out_all.T.reshape(B_TOTAL, HH * WW, C).transpose(0, 2, 1)
    return np.ascontiguousarray(out.reshape(B_TOTAL, C, HH, WW)).astype(np.float32)



# revision 8
# speedup vs baseline: 1.4678x; 1.4678x over previous
"""Bilinear RoI pooling, V4: overlap-tiled gather-matmul, slot-dealt cells.

The fp16 feature map lives in SBUF as per-(band, x-block) tiles

    T[v,b][p = yp*64 + xo, c] = feats[y = v + yp, x = 63*b + xo, c]

(v = 0..62 band = base row y0, yp in {0,1}, b = x-block 0..3 of width 63,
x-tile width 64 so x0+1 stays in-tile).  Every sample (one output pixel of
one RoI) is a 4-hot fp16 column against exactly ONE tile: weights at
partitions (xo, xo+1, 64+xo, 64+xo+1).  One matmul per (tile, channel
chunk) computes psum[c, s] = sum_k w_k[s] * feats[y_k, x_k, c] -- no
accumulation chains, no odd/even double pass (rows are stored twice
instead: ~4.2 MB/core).

x >= 252 ("tail") samples use a packed tile holding 16 band-pairs x 4
x-columns in the 128 partitions, so the whole tail is a handful of
matmuls.

Work distribution: the 63x4 (band, block) cells (large ones split into
<=1000-column pieces) are LPT-dealt to the 8 cores; each core packs its
pieces into shared schedule slots sorted by size, so per-slot quotas
(max over cores) give ONE static graph with ~2% padding.  The whole rhs
([128, ncol] fp16, 4-hot columns) is loaded once and stays resident.

PSUM is drained by BOTH the DVE and the Activation engine (alternating
256-column supers), cast fp32->fp16 into per-engine store rings, and
written out in 4-super batches.  Per-core HBM traffic ~37 MB
(4.3 F4 + 6.6 rhs + 26 stores) vs ~57 MB for V3."""

import hashlib
import heapq

import numpy as np

HH, WW = 7, 7
C, Hf, Wf = 512, 64, 256
NPY, NPX = Hf - 1, Wf - 1         # base grids: y0 in 0..62, x0 in 0..254
N_CORES = 8
B_TOTAL = 4096
S_CORE = B_TOTAL * HH * WW // N_CORES   # 25088 samples per core
XBW = 63                          # x-block width (blocks 0..3; x>=252 = tail)
NBLK = 4
SPLIT_MAX = 1000                  # max columns per schedule slot (cell piece)
SUP = 256                         # psum super columns
NRING = 16                        # store-ring buffers per cast engine
STB = 4                           # supers per store DMA batch
F4CH = 4                          # slots per f4 load chunk
RSLAB_N = 8                       # rhs load slabs

_NC_CACHE = {}


def _build_nc(plan):
    import concourse.bacc as bacc
    import concourse.mybir as mybir

    quota = [int(q) for q in plan["quota"]]          # per-slot columns
    quota_tail = int(plan["quota_tail"])
    n_slots = len(quota)
    ncol = sum(quota) + quota_tail
    assert ncol % SUP == 0
    n_supers = ncol // SUP
    # cast owner per super: even -> DVE, odd -> ACT
    own = [s % 2 for s in range(n_supers)]
    dve_list = [s for s in range(n_supers) if own[s] == 0]
    act_list = [s for s in range(n_supers) if own[s] == 1]
    s2stream = {}
    for j, s in enumerate(dve_list):
        s2stream[s] = (0, j)
    for j, s in enumerate(act_list):
        s2stream[s] = (1, j)
    nb = [(len(dve_list) + STB - 1) // STB, (len(act_list) + STB - 1) // STB]
    n_f4ch = (n_slots + F4CH - 1) // F4CH
    rslab = ((ncol + RSLAB_N - 1) // RSLAB_N + SUP - 1) // SUP * SUP

    # ---- static matmul schedule: (slot|'T', q, a, b, super) ----
    # segment-major: all 4 channel chunks of a super-segment before the
    # next segment, so supers close strictly in column order (a slot may
    # span many supers; q-major would open block s%4 before closing s-4).
    sched = []
    col = 0
    for l in [*range(n_slots), "T"]:
        hi = ncol if l == "T" else col + quota[l]
        a = col
        while a < hi:
            s = a // SUP
            b = min(hi, (s + 1) * SUP)
            for q in range(4):
                sched.append((l, q, a, b, s))
            a = b
        col = hi
    first_touch = {}
    last_touch = {}
    for i, ins in enumerate(sched):
        s = ins[4]
        first_touch.setdefault(s, i)
        last_touch[s] = i
    assert set(first_touch) == set(range(n_supers)), "super coverage hole"
    lt = [last_touch[s] for s in range(n_supers)]
    assert lt == sorted(lt), "non-monotone super retirement"
    inc_at = {i: s for s, i in last_touch.items()}
    wait_at = {i: s for s, i in first_touch.items()}

    nc = bacc.Bacc("TRN2", debug=False)
    f16, f32 = mybir.dt.float16, mybir.dt.float32

    f4_d = nc.dram_tensor("f4", [128, n_slots * C], f16, kind="ExternalInput")
    tl_d = nc.dram_tensor("tl", [128, C], f16, kind="ExternalInput")
    rhs_d = nc.dram_tensor("rhs", [128, ncol], f16, kind="ExternalInput")
    out_d = [
        nc.dram_tensor(nm, [nbk, 128, STB * 1024], f16, kind="ExternalOutput")
        for nm, nbk in (("out_dve", nb[0]), ("out_act", nb[1]))
    ]

    f4 = nc.alloc_sbuf_tensor("f4_sb", [128, n_slots * C], f16)
    tl = nc.alloc_sbuf_tensor("tl_sb", [128, C], f16)
    rhs = nc.alloc_sbuf_tensor("rhs_sb", [128, ncol], f16)
    st = [
        nc.alloc_sbuf_tensor(f"st{e}", [128, NRING * 1024], f16) for e in range(2)
    ]
    ps = nc.alloc_psum_tensor("ps", [128, 4096], f32)

    f_sems = [nc.alloc_semaphore(f"f_sem{i}") for i in range(n_f4ch)]
    t_sem = nc.alloc_semaphore("t_sem")
    r_sems = [nc.alloc_semaphore(f"r_sem{i}") for i in range(RSLAB_N)]
    pe_sem = nc.alloc_semaphore("pe_sem")
    cast_sems = [nc.alloc_semaphore(f"cast_sem{e}") for e in range(2)]
    # one sem per store-ring position: same-position stores are serialized
    # by the ring-reuse cast gating, so thresholds are unambiguous even
    # with out-of-order DMA completions across positions.
    NPOS = NRING // STB
    st_sems = [
        [nc.alloc_semaphore(f"st_sem{e}_{p}") for p in range(NPOS)]
        for e in range(2)
    ]

    def cast_wait_for(engine, s):
        """Wait until super s is cast (psum block reusable)."""
        e, j = s2stream[s]
        engine.wait_ge(cast_sems[e], j + 1)

    def emit_cast(engine, e, s, j, copy):
        engine.wait_ge(pe_sem, s + 1)
        if j >= NRING:
            k_need = j // STB - NPOS          # store batch freeing this buf
            engine.wait_ge(st_sems[e][k_need % NPOS], 16 * (k_need // NPOS + 1))
        dst = st[e][:, (j % NRING) * 1024 : (j % NRING) * 1024 + 1024]
        src = ps[:, (s % 4) * 1024 : (s % 4) * 1024 + 1024]
        copy(dst, src).then_inc(cast_sems[e], 1)

    with nc.Block() as block:

        @block.scalar
        def _(scalar):
            # interleave f4 chunks / rhs slabs so the PE's consumption
            # order (slot-major) is fed first; then this engine casts.
            nch = max(n_f4ch, RSLAB_N)
            for k in range(nch):
                if k < n_f4ch:
                    c0 = k * F4CH * C
                    c1 = min(n_slots * C, (k + 1) * F4CH * C)
                    scalar.dma_start(
                        f4[:, c0:c1], f4_d[:, c0:c1]
                    ).then_inc(f_sems[k], 16)
                if k == 0:
                    scalar.dma_start(tl[:, :], tl_d[:, :]).then_inc(t_sem, 16)
                if k < RSLAB_N:
                    a = k * rslab
                    b = min(ncol, (k + 1) * rslab)
                    if b > a:
                        scalar.dma_start(
                            rhs[:, a:b], rhs_d[:, a:b]
                        ).then_inc(r_sems[k], 16)
            for j, s in enumerate(act_list):
                emit_cast(scalar, 1, s, j, scalar.copy)

        @block.vector
        def _(vector):
            for j, s in enumerate(dve_list):
                emit_cast(vector, 0, s, j, vector.tensor_copy)

        @block.tensor
        def _(tensor):
            seen_slot = set()
            rmax = [0]
            tensor.wait_ge(t_sem, 16)
            for i, (l, q, a, b, s) in enumerate(sched):
                if l not in seen_slot:
                    seen_slot.add(l)
                    if l != "T":
                        tensor.wait_ge(f_sems[l // F4CH], 16)
                rs = (b - 1) // rslab
                if rs >= rmax[0]:
                    for k in range(rmax[0], rs + 1):
                        tensor.wait_ge(r_sems[k], 16)
                    rmax[0] = rs + 1
                if i in wait_at and wait_at[i] >= 4:
                    cast_wait_for(tensor, wait_at[i] - 4)
                if l == "T":
                    lhsT = tl[:, 128 * q : 128 * (q + 1)]
                else:
                    lhsT = f4[:, l * C + 128 * q : l * C + 128 * (q + 1)]
                off = (s % 4) * 1024 + q * SUP + (a - s * SUP)
                mm = tensor.matmul(
                    ps[:, off : off + (b - a)],
                    lhsT,
                    rhs[:, a:b],
                    start=True,
                    stop=True,
                    skip_group_check=True,
                )
                if i in inc_at:
                    mm.then_inc(pe_sem, 1)

        @block.sync
        def _(sync):
            # all stores, both streams, ordered by global super time
            batches = []
            for e in range(2):
                n_e = len((dve_list, act_list)[e])
                for k in range(nb[e]):
                    last_s = ((dve_list, act_list)[e])[min(4 * k + 3, n_e - 1)]
                    batches.append((last_s, e, k, min(4 * k + 4, n_e)))
            batches.sort()
            for _, e, k, cth in batches:
                sync.wait_ge(cast_sems[e], cth)
                r0 = (4 * k % NRING) * 1024
                sync.dma_start(
                    out_d[e][k], st[e][:, r0 : r0 + STB * 1024]
                ).then_inc(st_sems[e][k % NPOS], 16)
            for e in range(2):
                for p in range(NPOS):
                    cnt = (nb[e] - p + NPOS - 1) // NPOS if nb[e] > p else 0
                    if cnt:
                        sync.wait_ge(st_sems[e][p], 16 * cnt)

    nc.compile()
    return nc


def _get_nc(plan):
    key = hashlib.sha256(
        np.asarray(plan["quota"], np.int64).tobytes()
        + np.int64(plan["quota_tail"]).tobytes()
    ).hexdigest()
    if key not in _NC_CACHE:
        _NC_CACHE[key] = _build_nc(plan)
    return _NC_CACHE[key]


def _host_prep(feats, boxes, img_height, img_width):
    """Per-sample base row (y0*255 + x0, clamped) and 4 slot weights
    (tl, tr, bl, br with validity and clamp-aggregation folded in),
    mirroring the reference math in f32."""
    B = boxes.shape[0]
    f32 = np.float32
    xc, yc, w, h = (boxes[:, k].astype(f32) for k in range(4))
    tx = np.linspace(-1.0, 1.0, WW, dtype=f32)
    ty = np.linspace(-1.0, 1.0, HH, dtype=f32)
    inv_w = f32(1.0) / f32(img_width - 1)
    inv_h = f32(1.0) / f32(img_height - 1)
    gx = (f32(2.0) * xc[:, None] - f32(img_width - 1)) * inv_w \
        + (w * inv_w)[:, None] * tx[None, :]
    gy = (f32(2.0) * yc[:, None] - f32(img_height - 1)) * inv_h \
        + (h * inv_h)[:, None] * ty[None, :]
    px = (gx + f32(1.0)) * f32(0.5) * f32(Wf - 1)   # (B, WW)
    py = (gy + f32(1.0)) * f32(0.5) * f32(Hf - 1)   # (B, HH)

    x0 = np.floor(px)
    y0 = np.floor(py)
    fx, fy = px - x0, py - y0
    x0i, y0i = x0.astype(np.int64), y0.astype(np.int64)
    x1i, y1i = x0i + 1, y0i + 1
    vx0 = ((x0i >= 0) & (x0i <= Wf - 1)).astype(f32)
    vx1 = ((x1i >= 0) & (x1i <= Wf - 1)).astype(f32)
    vy0 = ((y0i >= 0) & (y0i <= Hf - 1)).astype(f32)
    vy1 = ((y1i >= 0) & (y1i <= Hf - 1)).astype(f32)
    x0c = np.clip(x0i, 0, Wf - 1).astype(np.int32)
    x1c = np.clip(x1i, 0, Wf - 1).astype(np.int32)
    y0c = np.clip(y0i, 0, Hf - 1).astype(np.int32)
    y1c = np.clip(y1i, 0, Hf - 1).astype(np.int32)

    def by(a):
        return np.broadcast_to(a[:, :, None], (B, HH, WW))

    def bx(a):
        return np.broadcast_to(a[:, None, :], (B, HH, WW))

    base_y = np.clip(y0i, 0, NPY - 1)                 # (B, HH)
    base_x = np.clip(x0i, 0, NPX - 1)                 # (B, WW)
    rows = (by(base_y) * NPX + bx(base_x)).reshape(-1).astype(np.int32)

    wx0, wx1 = f32(1.0) - fx, fx
    wy0, wy1 = f32(1.0) - fy, fy
    wk = np.stack(
        [
            by(wy0 * vy0) * bx(wx0 * vx0),
            by(wy0 * vy0) * bx(wx1 * vx1),
            by(wy1 * vy1) * bx(wx0 * vx0),
            by(wy1 * vy1) * bx(wx1 * vx1),
        ],
        axis=-1,
    ).reshape(B * HH * WW, 4).astype(f32)
    dy = np.stack(
        [by(y0c - base_y), by(y0c - base_y), by(y1c - base_y), by(y1c - base_y)],
        axis=-1,
    ).reshape(B * HH * WW, 4)
    dx = np.stack(
        [bx(x0c - base_x), bx(x1c - base_x), bx(x0c - base_x), bx(x1c - base_x)],
        axis=-1,
    ).reshape(B * HH * WW, 4)
    slots = np.clip(dy, 0, 1) * 2 + np.clip(dx, 0, 1)
    wts = np.zeros((B * HH * WW, 4), f32)
    np.add.at(wts, (np.arange(B * HH * WW)[:, None], slots), wk)
    return rows, wts


def _prepare(feats, boxes, img_height, img_width):
    rows, wts = _host_prep(feats, boxes, img_height, img_width)
    n = rows.shape[0]
    y0 = (rows // NPX).astype(np.int64)          # 0..62
    x0 = (rows % NPX).astype(np.int64)           # 0..254
    blk = np.minimum(x0 // XBW, NBLK)            # 0..4 (4 = tail)
    is_tail = blk == NBLK

    # per-cell sample id lists (stable order)
    cell_of = y0 * (NBLK + 1) + blk
    order = np.argsort(cell_of, kind="stable")
    co = cell_of[order]
    starts = np.r_[0, np.flatnonzero(co[1:] != co[:-1]) + 1]
    uniq = co[starts]
    lens = np.diff(np.r_[starts, n])
    cell_ids = {int(u): order[s : s + L] for u, s, L in zip(uniq, starts, lens)}

    # split non-tail cells into pieces of <= SPLIT_MAX columns
    pieces = []                                   # (size, v, b, off)
    for v in range(NPY):
        for b in range(NBLK):
            ids = cell_ids.get(v * (NBLK + 1) + b)
            if ids is None:
                continue
            cnum = len(ids)
            k = -(-cnum // SPLIT_MAX)
            base, rem = cnum // k, cnum % k
            off = 0
            for j in range(k):
                sz = base + (1 if j < rem else 0)
                pieces.append((sz, v, b, off))
                off += sz
    pieces.sort(reverse=True)

    # LPT deal to cores; per-core lists stay size-sorted by re-sorting
    heap = [(0, 0, m) for m in range(N_CORES)]
    heapq.heapify(heap)
    percore = [[] for _ in range(N_CORES)]
    for p in pieces:
        tot, ns, m = heapq.heappop(heap)
        percore[m].append(p)
        heapq.heappush(heap, (tot + p[0], ns + 1, m))
    for m in range(N_CORES):
        percore[m].sort(reverse=True)
    n_slots = max(len(p) for p in percore)
    quota = np.zeros(n_slots, np.int64)
    for m in range(N_CORES):
        for l, p in enumerate(percore[m]):
            quota[l] = max(quota[l], p[0])

    # tail cells: snake-deal by size (<= 16 per core)
    tcells = sorted(
        (
            (len(cell_ids[v * (NBLK + 1) + NBLK]), v)
            for v in range(NPY)
            if v * (NBLK + 1) + NBLK in cell_ids
        ),
        reverse=True,
    )
    tcore = [[] for _ in range(N_CORES)]
    ttot = np.zeros(N_CORES, np.int64)
    for i, c in enumerate(tcells):
        r, m = divmod(i, N_CORES)
        m = m if r % 2 == 0 else N_CORES - 1 - m
        tcore[m].append(c)
        ttot[m] += c[0]
    assert max(len(t) for t in tcore) <= 16, "tail tile overflow"
    qt = int(ttot.max())
    ncol = int(quota.sum()) + qt
    qt += (-ncol) % SUP
    ncol += (-ncol) % SUP
    plan = {"quota": quota, "quota_tail": qt, "ncol": ncol}

    slot_start = np.zeros(n_slots + 1, np.int64)
    np.cumsum(quota, out=slot_start[1:])
    tail_start = int(slot_start[n_slots])

    ftab = feats.astype(np.float16)               # (C, Hf, Wf)
    yp_ = np.arange(128) // 64
    xo_ = np.arange(128) % 64

    in_maps, colmaps = [], []
    for m in range(N_CORES):
        f4_dat = np.zeros((128, n_slots * C), np.float16)
        rhs = np.zeros((128, ncol), np.float16)
        colmap = np.full(ncol, -1, np.int64)
        for l, (sz, v, b, off) in enumerate(percore[m]):
            ids = cell_ids[v * (NBLK + 1) + b][off : off + sz]
            # tile: [p = yp*64 + xo, c] = feats[v + yp, 63*b + xo, c]
            f4_dat[:, l * C : (l + 1) * C] = ftab[
                :, v + yp_, XBW * b + xo_
            ].T
            cols = slot_start[l] + np.arange(sz)
            colmap[cols] = ids
            xo = x0[ids] - XBW * b
            w4 = wts[ids]
            rhs[xo, cols] = w4[:, 0]
            rhs[xo + 1, cols] = w4[:, 1]
            rhs[64 + xo, cols] = w4[:, 2]
            rhs[64 + xo + 1, cols] = w4[:, 3]
        # tail tile: [p = u*8 + yp*4 + xoff, c] = feats[v_u + yp, 252 + xoff, c]
        tl_dat = np.zeros((128, C), np.float16)
        tcol = tail_start
        for u, (sz, v) in enumerate(tcore[m]):
            p_ = np.arange(8)
            tl_dat[u * 8 + p_] = ftab[
                :, v + p_ // 4, np.minimum(NBLK * XBW + p_ % 4, Wf - 1)
            ].T
            ids = cell_ids[v * (NBLK + 1) + NBLK]
            cols = tcol + np.arange(sz)
            tcol += sz
            colmap[cols] = ids
            xoff = x0[ids] - NBLK * XBW
            w4 = wts[ids]
            rhs[u * 8 + xoff, cols] = w4[:, 0]
            rhs[u * 8 + xoff + 1, cols] = w4[:, 1]
            rhs[u * 8 + 4 + xoff, cols] = w4[:, 2]
            rhs[u * 8 + 4 + xoff + 1, cols] = w4[:, 3]
        in_maps.append(
            {
                "f4": f4_dat,
                "tl": tl_dat,
                "rhs": np.ascontiguousarray(rhs),
            }
        )
        colmaps.append(colmap)

    return plan, in_maps, colmaps


def kernel(**inputs):
    from concourse.bass_utils import run_bass_kernel_spmd

    feats = np.asarray(inputs["feats"], dtype=np.float32)
    boxes = np.asarray(inputs["boxes"], dtype=np.float32)
    img_height = int(np.asarray(inputs["img_height"]))
    img_width = int(np.asarray(inputs["img_width"]))

    plan, in_maps, colmaps = _prepare(feats, boxes, img_height, img_width)
    nc = _get_nc(plan)
    res = run_bass_kernel_spmd(nc, in_maps, core_ids=list(range(N_CORES)))

    ncol = plan["ncol"]
    n_supers = ncol // SUP
    out_all = np.empty((C, B_TOTAL * HH * WW), np.float32)
    for m, r in enumerate(res.results):
        full = np.empty((C, ncol), np.float32)
        for e, nm in enumerate(("out_dve", "out_act")):
            a = r[nm]                              # (nb, 128, 4*1024) f16
            slist = [s for s in range(n_supers) if s % 2 == e]
            nbk = a.shape[0]
            # a[k, p, j*1024 + q*256 + r] -> [q*128 + p, (k*STB + j)*SUP + r]
            x = (
                a.reshape(nbk, 128, STB, 4, SUP)
                .transpose(3, 1, 0, 2, 4)
                .reshape(C, nbk * STB * SUP)
            )
            for j, s in enumerate(slist):
                full[:, s * SUP : (s + 1) * SUP] = x[
                    :, j * SUP : (j + 1) * SUP
                ].astype(np.float32)
        cm = colmaps[m]
        valid = cm >= 0
        out_all[:, cm[valid]] = full[:, valid]
    out = out_all.T.reshape(B_TOTAL, HH * WW, C).transpose(0, 2, 1)
    return np.ascontiguousarray(out.reshape(B_TOTAL, C, HH, WW)).astype(np.float32)


# revision 14
# speedup vs baseline: 1.4895x; 1.0148x over previous
"""Bilinear RoI pooling, V4: overlap-tiled gather-matmul, slot-dealt cells.

The fp16 feature map lives in SBUF as per-(band, x-block) tiles

    T[v,b][p = yp*64 + xo, c] = feats[y = v + yp, x = 63*b + xo, c]

(v = 0..62 band = base row y0, yp in {0,1}, b = x-block 0..3 of width 63,
x-tile width 64 so x0+1 stays in-tile).  Every sample (one output pixel of
one RoI) is a 4-hot fp16 column against exactly ONE tile: weights at
partitions (xo, xo+1, 64+xo, 64+xo+1).  One matmul per (tile, channel
chunk) computes psum[c, s] = sum_k w_k[s] * feats[y_k, x_k, c] -- no
accumulation chains, no odd/even double pass (rows are stored twice
instead: ~4.2 MB/core).

x >= 252 ("tail") samples use a packed tile holding 16 band-pairs x 4
x-columns in the 128 partitions, so the whole tail is a handful of
matmuls.

Work distribution: the 63x4 (band, block) cells (large ones split into
<=1000-column pieces) are LPT-dealt to the 8 cores; each core packs its
pieces into shared schedule slots sorted by size, so per-slot quotas
(max over cores) give ONE static graph with ~2% padding.  The whole rhs
([128, ncol] fp16, 4-hot columns) is loaded once and stays resident.

PSUM is drained by BOTH the DVE and the Activation engine (alternating
256-column supers), cast fp32->fp16 into per-engine store rings, and
written out in 4-super batches.  Per-core HBM traffic ~37 MB
(4.3 F4 + 6.6 rhs + 26 stores) vs ~57 MB for V3."""

import hashlib
import heapq

import numpy as np

HH, WW = 7, 7
C, Hf, Wf = 512, 64, 256
NPY, NPX = Hf - 1, Wf - 1         # base grids: y0 in 0..62, x0 in 0..254
N_CORES = 8
B_TOTAL = 4096
S_CORE = B_TOTAL * HH * WW // N_CORES   # 25088 samples per core
XBW = 63                          # x-block width (blocks 0..3; x>=252 = tail)
NBLK = 4
SPLIT_MAX = 1000                  # max columns per schedule slot (cell piece)
SUP = 256                         # psum super columns
NRING = 16                        # store-ring buffers per cast engine
STB = 2                           # supers per store DMA batch
RSLAB_N = 8                       # rhs load slabs

_NC_CACHE = {}


def _build_nc(plan):
    import concourse.bacc as bacc
    import concourse.mybir as mybir

    quota = [int(q) for q in plan["quota"]]          # per-slot columns
    quota_tail = int(plan["quota_tail"])
    n_slots = len(quota)
    ncol = sum(quota) + quota_tail
    assert ncol % SUP == 0
    n_supers = ncol // SUP
    # cast owner per super: even -> DVE, odd -> ACT
    own = [s % 2 for s in range(n_supers)]
    dve_list = [s for s in range(n_supers) if own[s] == 0]
    act_list = [s for s in range(n_supers) if own[s] == 1]
    s2stream = {}
    for j, s in enumerate(dve_list):
        s2stream[s] = (0, j)
    for j, s in enumerate(act_list):
        s2stream[s] = (1, j)
    nb = [(len(dve_list) + STB - 1) // STB, (len(act_list) + STB - 1) // STB]
    # f4 load chunks: small leading chunks so the PE can start early
    fbounds = [0, 2, 4]
    while fbounds[-1] < n_slots:
        fbounds.append(min(n_slots, fbounds[-1] + 4))
    n_f4ch = len(fbounds) - 1
    slot_chunk = {}
    for k in range(n_f4ch):
        for l in range(fbounds[k], fbounds[k + 1]):
            slot_chunk[l] = k
    # rhs load slabs: small leading slab, then even 256-aligned splits
    rs = ((ncol - 1024 + (RSLAB_N - 2)) // (RSLAB_N - 1) + SUP - 1) // SUP * SUP
    rbounds = [0, 1024]
    while rbounds[-1] < ncol:
        rbounds.append(min(ncol, rbounds[-1] + rs))
    n_rslab = len(rbounds) - 1

    # ---- static matmul schedule: (slot|'T', q, a, b, super) ----
    # segment-major: all 4 channel chunks of a super-segment before the
    # next segment, so supers close strictly in column order (a slot may
    # span many supers; q-major would open block s%4 before closing s-4).
    sched = []
    col = 0
    for l in [*range(n_slots), "T"]:
        hi = ncol if l == "T" else col + quota[l]
        a = col
        while a < hi:
            s = a // SUP
            b = min(hi, (s + 1) * SUP)
            for q in range(4):
                sched.append((l, q, a, b, s))
            a = b
        col = hi
    first_touch = {}
    last_touch = {}
    for i, ins in enumerate(sched):
        s = ins[4]
        first_touch.setdefault(s, i)
        last_touch[s] = i
    assert set(first_touch) == set(range(n_supers)), "super coverage hole"
    lt = [last_touch[s] for s in range(n_supers)]
    assert lt == sorted(lt), "non-monotone super retirement"
    inc_at = {i: s for s, i in last_touch.items()}
    wait_at = {i: s for s, i in first_touch.items()}

    nc = bacc.Bacc("TRN2", debug=False)
    f16, f32 = mybir.dt.float16, mybir.dt.float32

    f4_d = nc.dram_tensor("f4", [128, n_slots * C], f16, kind="ExternalInput")
    tl_d = nc.dram_tensor("tl", [128, C], f16, kind="ExternalInput")
    rhs_d = nc.dram_tensor("rhs", [128, ncol], f16, kind="ExternalInput")
    out_d = [
        nc.dram_tensor(nm, [nbk, 128, STB * 1024], f16, kind="ExternalOutput")
        for nm, nbk in (("out_dve", nb[0]), ("out_act", nb[1]))
    ]

    f4 = nc.alloc_sbuf_tensor("f4_sb", [128, n_slots * C], f16)
    tl = nc.alloc_sbuf_tensor("tl_sb", [128, C], f16)
    rhs = nc.alloc_sbuf_tensor("rhs_sb", [128, ncol], f16)
    st = [
        nc.alloc_sbuf_tensor(f"st{e}", [128, NRING * 1024], f16) for e in range(2)
    ]
    ps = nc.alloc_psum_tensor("ps", [128, 4096], f32)

    f_sems = [nc.alloc_semaphore(f"f_sem{i}") for i in range(n_f4ch)]
    t_sem = nc.alloc_semaphore("t_sem")
    r_sems = [nc.alloc_semaphore(f"r_sem{i}") for i in range(n_rslab)]
    pe_sem = nc.alloc_semaphore("pe_sem")
    cast_sems = [nc.alloc_semaphore(f"cast_sem{e}") for e in range(2)]
    # one sem per store-ring position: same-position stores are serialized
    # by the ring-reuse cast gating, so thresholds are unambiguous even
    # with out-of-order DMA completions across positions.
    NPOS = NRING // STB
    st_sems = [
        [nc.alloc_semaphore(f"st_sem{e}_{p}") for p in range(NPOS)]
        for e in range(2)
    ]

    def cast_wait_for(engine, s):
        """Wait until super s is cast (psum block reusable)."""
        e, j = s2stream[s]
        engine.wait_ge(cast_sems[e], j + 1)

    def emit_cast(engine, e, s, j, copy):
        engine.wait_ge(pe_sem, s + 1)
        if j >= NRING:
            k_need = j // STB - NPOS          # store batch freeing this buf
            engine.wait_ge(st_sems[e][k_need % NPOS], 16 * (k_need // NPOS + 1))
        dst = st[e][:, (j % NRING) * 1024 : (j % NRING) * 1024 + 1024]
        src = ps[:, (s % 4) * 1024 : (s % 4) * 1024 + 1024]
        copy(dst, src).then_inc(cast_sems[e], 1)

    with nc.Block() as block:

        @block.scalar
        def _(scalar):
            # interleave f4 chunks / rhs slabs so the PE's consumption
            # order (slot-major) is fed first; then this engine casts.
            nch = max(n_f4ch, n_rslab)
            for k in range(nch):
                if k < n_f4ch:
                    c0, c1 = fbounds[k] * C, fbounds[k + 1] * C
                    scalar.dma_start(
                        f4[:, c0:c1], f4_d[:, c0:c1]
                    ).then_inc(f_sems[k], 16)
                if k < n_rslab:
                    a, b = rbounds[k], rbounds[k + 1]
                    scalar.dma_start(
                        rhs[:, a:b], rhs_d[:, a:b]
                    ).then_inc(r_sems[k], 16)
                if k == 2:
                    scalar.dma_start(tl[:, :], tl_d[:, :]).then_inc(t_sem, 16)
            for j, s in enumerate(act_list):
                emit_cast(scalar, 1, s, j, scalar.copy)

        @block.vector
        def _(vector):
            for j, s in enumerate(dve_list):
                emit_cast(vector, 0, s, j, vector.tensor_copy)

        @block.tensor
        def _(tensor):
            seen_slot = set()
            rmax = [0]
            for i, (l, q, a, b, s) in enumerate(sched):
                if l not in seen_slot:
                    seen_slot.add(l)
                    if l == "T":
                        tensor.wait_ge(t_sem, 16)
                    else:
                        tensor.wait_ge(f_sems[slot_chunk[l]], 16)
                while rmax[0] < n_rslab and rbounds[rmax[0]] < b:
                    tensor.wait_ge(r_sems[rmax[0]], 16)
                    rmax[0] += 1
                if i in wait_at and wait_at[i] >= 4:
                    cast_wait_for(tensor, wait_at[i] - 4)
                if l == "T":
                    lhsT = tl[:, 128 * q : 128 * (q + 1)]
                else:
                    lhsT = f4[:, l * C + 128 * q : l * C + 128 * (q + 1)]
                off = (s % 4) * 1024 + q * SUP + (a - s * SUP)
                mm = tensor.matmul(
                    ps[:, off : off + (b - a)],
                    lhsT,
                    rhs[:, a:b],
                    start=True,
                    stop=True,
                    skip_group_check=True,
                )
                if i in inc_at:
                    mm.then_inc(pe_sem, 1)

        @block.sync
        def _(sync):
            # all stores, both streams, ordered by global super time
            batches = []
            for e in range(2):
                n_e = len((dve_list, act_list)[e])
                for k in range(nb[e]):
                    last_s = ((dve_list, act_list)[e])[min(STB * (k + 1) - 1, n_e - 1)]
                    batches.append((last_s, e, k, min(STB * (k + 1), n_e)))
            batches.sort()
            for _, e, k, cth in batches:
                sync.wait_ge(cast_sems[e], cth)
                r0 = (STB * k % NRING) * 1024
                sync.dma_start(
                    out_d[e][k], st[e][:, r0 : r0 + STB * 1024]
                ).then_inc(st_sems[e][k % NPOS], 16)
            for e in range(2):
                for p in range(NPOS):
                    cnt = (nb[e] - p + NPOS - 1) // NPOS if nb[e] > p else 0
                    if cnt:
                        sync.wait_ge(st_sems[e][p], 16 * cnt)

    nc.compile()
    return nc


def _get_nc(plan):
    key = hashlib.sha256(
        np.asarray(plan["quota"], np.int64).tobytes()
        + np.int64(plan["quota_tail"]).tobytes()
    ).hexdigest()
    if key not in _NC_CACHE:
        _NC_CACHE[key] = _build_nc(plan)
    return _NC_CACHE[key]


def _host_prep(feats, boxes, img_height, img_width):
    """Per-sample base row (y0*255 + x0, clamped) and 4 slot weights
    (tl, tr, bl, br with validity and clamp-aggregation folded in),
    mirroring the reference math in f32."""
    B = boxes.shape[0]
    f32 = np.float32
    xc, yc, w, h = (boxes[:, k].astype(f32) for k in range(4))
    tx = np.linspace(-1.0, 1.0, WW, dtype=f32)
    ty = np.linspace(-1.0, 1.0, HH, dtype=f32)
    inv_w = f32(1.0) / f32(img_width - 1)
    inv_h = f32(1.0) / f32(img_height - 1)
    gx = (f32(2.0) * xc[:, None] - f32(img_width - 1)) * inv_w \
        + (w * inv_w)[:, None] * tx[None, :]
    gy = (f32(2.0) * yc[:, None] - f32(img_height - 1)) * inv_h \
        + (h * inv_h)[:, None] * ty[None, :]
    px = (gx + f32(1.0)) * f32(0.5) * f32(Wf - 1)   # (B, WW)
    py = (gy + f32(1.0)) * f32(0.5) * f32(Hf - 1)   # (B, HH)

    x0 = np.floor(px)
    y0 = np.floor(py)
    fx, fy = px - x0, py - y0
    x0i, y0i = x0.astype(np.int64), y0.astype(np.int64)
    x1i, y1i = x0i + 1, y0i + 1
    vx0 = ((x0i >= 0) & (x0i <= Wf - 1)).astype(f32)
    vx1 = ((x1i >= 0) & (x1i <= Wf - 1)).astype(f32)
    vy0 = ((y0i >= 0) & (y0i <= Hf - 1)).astype(f32)
    vy1 = ((y1i >= 0) & (y1i <= Hf - 1)).astype(f32)
    x0c = np.clip(x0i, 0, Wf - 1).astype(np.int32)
    x1c = np.clip(x1i, 0, Wf - 1).astype(np.int32)
    y0c = np.clip(y0i, 0, Hf - 1).astype(np.int32)
    y1c = np.clip(y1i, 0, Hf - 1).astype(np.int32)

    def by(a):
        return np.broadcast_to(a[:, :, None], (B, HH, WW))

    def bx(a):
        return np.broadcast_to(a[:, None, :], (B, HH, WW))

    base_y = np.clip(y0i, 0, NPY - 1)                 # (B, HH)
    base_x = np.clip(x0i, 0, NPX - 1)                 # (B, WW)
    rows = (by(base_y) * NPX + bx(base_x)).reshape(-1).astype(np.int32)

    wx0, wx1 = f32(1.0) - fx, fx
    wy0, wy1 = f32(1.0) - fy, fy
    wk = np.stack(
        [
            by(wy0 * vy0) * bx(wx0 * vx0),
            by(wy0 * vy0) * bx(wx1 * vx1),
            by(wy1 * vy1) * bx(wx0 * vx0),
            by(wy1 * vy1) * bx(wx1 * vx1),
        ],
        axis=-1,
    ).reshape(B * HH * WW, 4).astype(f32)
    dy = np.stack(
        [by(y0c - base_y), by(y0c - base_y), by(y1c - base_y), by(y1c - base_y)],
        axis=-1,
    ).reshape(B * HH * WW, 4)
    dx = np.stack(
        [bx(x0c - base_x), bx(x1c - base_x), bx(x0c - base_x), bx(x1c - base_x)],
        axis=-1,
    ).reshape(B * HH * WW, 4)
    slots = np.clip(dy, 0, 1) * 2 + np.clip(dx, 0, 1)
    wts = np.zeros((B * HH * WW, 4), f32)
    np.add.at(wts, (np.arange(B * HH * WW)[:, None], slots), wk)
    return rows, wts


def _prepare(feats, boxes, img_height, img_width):
    rows, wts = _host_prep(feats, boxes, img_height, img_width)
    n = rows.shape[0]
    y0 = (rows // NPX).astype(np.int64)          # 0..62
    x0 = (rows % NPX).astype(np.int64)           # 0..254
    blk = np.minimum(x0 // XBW, NBLK)            # 0..4 (4 = tail)
    is_tail = blk == NBLK

    # per-cell sample id lists (stable order)
    cell_of = y0 * (NBLK + 1) + blk
    order = np.argsort(cell_of, kind="stable")
    co = cell_of[order]
    starts = np.r_[0, np.flatnonzero(co[1:] != co[:-1]) + 1]
    uniq = co[starts]
    lens = np.diff(np.r_[starts, n])
    cell_ids = {int(u): order[s : s + L] for u, s, L in zip(uniq, starts, lens)}

    # split non-tail cells into pieces of <= SPLIT_MAX columns
    pieces = []                                   # (size, v, b, off)
    for v in range(NPY):
        for b in range(NBLK):
            ids = cell_ids.get(v * (NBLK + 1) + b)
            if ids is None:
                continue
            cnum = len(ids)
            k = -(-cnum // SPLIT_MAX)
            base, rem = cnum // k, cnum % k
            off = 0
            for j in range(k):
                sz = base + (1 if j < rem else 0)
                pieces.append((sz, v, b, off))
                off += sz
    pieces.sort(reverse=True)

    # LPT deal to cores; per-core lists stay size-sorted by re-sorting
    heap = [(0, 0, m) for m in range(N_CORES)]
    heapq.heapify(heap)
    percore = [[] for _ in range(N_CORES)]
    for p in pieces:
        tot, ns, m = heapq.heappop(heap)
        percore[m].append(p)
        heapq.heappush(heap, (tot + p[0], ns + 1, m))
    for m in range(N_CORES):
        percore[m].sort(reverse=True)
    n_slots = max(len(p) for p in percore)
    quota = np.zeros(n_slots, np.int64)
    for m in range(N_CORES):
        for l, p in enumerate(percore[m]):
            quota[l] = max(quota[l], p[0])

    # tail cells: snake-deal by size (<= 16 per core)
    tcells = sorted(
        (
            (len(cell_ids[v * (NBLK + 1) + NBLK]), v)
            for v in range(NPY)
            if v * (NBLK + 1) + NBLK in cell_ids
        ),
        reverse=True,
    )
    tcore = [[] for _ in range(N_CORES)]
    ttot = np.zeros(N_CORES, np.int64)
    for i, c in enumerate(tcells):
        r, m = divmod(i, N_CORES)
        m = m if r % 2 == 0 else N_CORES - 1 - m
        tcore[m].append(c)
        ttot[m] += c[0]
    assert max(len(t) for t in tcore) <= 16, "tail tile overflow"
    qt = int(ttot.max())
    ncol = int(quota.sum()) + qt
    qt += (-ncol) % SUP
    ncol += (-ncol) % SUP
    plan = {"quota": quota, "quota_tail": qt, "ncol": ncol}

    slot_start = np.zeros(n_slots + 1, np.int64)
    np.cumsum(quota, out=slot_start[1:])
    tail_start = int(slot_start[n_slots])

    ftab = feats.astype(np.float16)               # (C, Hf, Wf)
    yp_ = np.arange(128) // 64
    xo_ = np.arange(128) % 64

    in_maps, colmaps = [], []
    for m in range(N_CORES):
        f4_dat = np.zeros((128, n_slots * C), np.float16)
        rhs = np.zeros((128, ncol), np.float16)
        colmap = np.full(ncol, -1, np.int64)
        for l, (sz, v, b, off) in enumerate(percore[m]):
            ids = cell_ids[v * (NBLK + 1) + b][off : off + sz]
            # tile: [p = yp*64 + xo, c] = feats[v + yp, 63*b + xo, c]
            f4_dat[:, l * C : (l + 1) * C] = ftab[
                :, v + yp_, XBW * b + xo_
            ].T
            cols = slot_start[l] + np.arange(sz)
            colmap[cols] = ids
            xo = x0[ids] - XBW * b
            w4 = wts[ids]
            rhs[xo, cols] = w4[:, 0]
            rhs[xo + 1, cols] = w4[:, 1]
            rhs[64 + xo, cols] = w4[:, 2]
            rhs[64 + xo + 1, cols] = w4[:, 3]
        # tail tile: [p = u*8 + yp*4 + xoff, c] = feats[v_u + yp, 252 + xoff, c]
        tl_dat = np.zeros((128, C), np.float16)
        tcol = tail_start
        for u, (sz, v) in enumerate(tcore[m]):
            p_ = np.arange(8)
            tl_dat[u * 8 + p_] = ftab[
                :, v + p_ // 4, np.minimum(NBLK * XBW + p_ % 4, Wf - 1)
            ].T
            ids = cell_ids[v * (NBLK + 1) + NBLK]
            cols = tcol + np.arange(sz)
            tcol += sz
            colmap[cols] = ids
            xoff = x0[ids] - NBLK * XBW
            w4 = wts[ids]
            rhs[u * 8 + xoff, cols] = w4[:, 0]
            rhs[u * 8 + xoff + 1, cols] = w4[:, 1]
            rhs[u * 8 + 4 + xoff, cols] = w4[:, 2]
            rhs[u * 8 + 4 + xoff + 1, cols] = w4[:, 3]
        in_maps.append(
            {
                "f4": f4_dat,
                "tl": tl_dat,
                "rhs": np.ascontiguousarray(rhs),
            }
        )
        colmaps.append(colmap)

    return plan, in_maps, colmaps


def kernel(**inputs):
    from concourse.bass_utils import run_bass_kernel_spmd

    feats = np.asarray(inputs["feats"], dtype=np.float32)
    boxes = np.asarray(inputs["boxes"], dtype=np.float32)
    img_height = int(np.asarray(inputs["img_height"]))
    img_width = int(np.asarray(inputs["img_width"]))

    plan, in_maps, colmaps = _prepare(feats, boxes, img_height, img_width)
    nc = _get_nc(plan)
    res = run_bass_kernel_spmd(nc, in_maps, core_ids=list(range(N_CORES)))

    ncol = plan["ncol"]
    n_supers = ncol // SUP
    out_all = np.empty((C, B_TOTAL * HH * WW), np.float32)
    for m, r in enumerate(res.results):
        full = np.empty((C, ncol), np.float32)
        for e, nm in enumerate(("out_dve", "out_act")):
            a = r[nm]                              # (nb, 128, 4*1024) f16
            slist = [s for s in range(n_supers) if s % 2 == e]
            nbk = a.shape[0]
            # a[k, p, j*1024 + q*256 + r] -> [q*128 + p, (k*STB + j)*SUP + r]
            x = (
                a.reshape(nbk, 128, STB, 4, SUP)
                .transpose(3, 1, 0, 2, 4)
                .reshape(C, nbk * STB * SUP)
            )
            for j, s in enumerate(slist):
                full[:, s * SUP : (s + 1) * SUP] = x[
                    :, j * SUP : (j + 1) * SUP
                ].astype(np.float32)
        cm = colmaps[m]
        valid = cm >= 0
        out_all[:, cm[valid]] = full[:, valid]
    out = out_all.T.reshape(B_TOTAL, HH * WW, C).transpose(0, 2, 1)
    return np.ascontiguousarray(out.reshape(B_TOTAL, C, HH, WW)).astype(np.float32)


# revision 15
# speedup vs baseline: 1.6710x; 1.1218x over previous
"""Bilinear RoI pooling, V4: overlap-tiled gather-matmul, slot-dealt cells.

The fp16 feature map lives in SBUF as per-(band, x-block) tiles

    T[v,b][p = yp*64 + xo, c] = feats[y = v + yp, x = 63*b + xo, c]

(v = 0..62 band = base row y0, yp in {0,1}, b = x-block 0..3 of width 63,
x-tile width 64 so x0+1 stays in-tile).  Every sample (one output pixel of
one RoI) is a 4-hot fp16 column against exactly ONE tile: weights at
partitions (xo, xo+1, 64+xo, 64+xo+1).  One matmul per (tile, channel
chunk) computes psum[c, s] = sum_k w_k[s] * feats[y_k, x_k, c] -- no
accumulation chains, no odd/even double pass (rows are stored twice
instead: ~4.2 MB/core).

x >= 252 ("tail") samples use a packed tile holding 16 band-pairs x 4
x-columns in the 128 partitions, so the whole tail is a handful of
matmuls.

Work distribution: the 63x4 (band, block) cells (large ones split into
<=1000-column pieces) are LPT-dealt to the 8 cores; each core packs its
pieces into shared schedule slots sorted by size, so per-slot quotas
(max over cores) give ONE static graph with ~2% padding.  The whole rhs
([128, ncol] fp16, 4-hot columns) is loaded once and stays resident.

PSUM is drained by BOTH the DVE and the Activation engine (alternating
256-column supers), cast fp32->fp16 into per-engine store rings, and
written out in 4-super batches.  Per-core HBM traffic ~37 MB
(4.3 F4 + 6.6 rhs + 26 stores) vs ~57 MB for V3."""

import hashlib
import heapq

import numpy as np

HH, WW = 7, 7
C, Hf, Wf = 512, 64, 256
NPY, NPX = Hf - 1, Wf - 1         # base grids: y0 in 0..62, x0 in 0..254
N_CORES = 8
B_TOTAL = 4096
S_CORE = B_TOTAL * HH * WW // N_CORES   # 25088 samples per core
XBW = 63                          # x-block width (blocks 0..3; x>=252 = tail)
NBLK = 4
SPLIT_MAX = 1000                  # max columns per schedule slot (cell piece)
SUP = 256                         # psum super columns
NRING = 16                        # store-ring buffers per cast engine
STB = 2                           # supers per store DMA batch
RSLAB_N = 8                       # rhs load slabs

_NC_CACHE = {}


def _build_nc(plan):
    import concourse.bacc as bacc
    import concourse.mybir as mybir

    quota = [int(q) for q in plan["quota"]]          # per-slot columns
    quota_tail = int(plan["quota_tail"])
    n_slots = len(quota)
    ncol = sum(quota) + quota_tail
    assert ncol % SUP == 0
    n_supers = ncol // SUP
    # cast owner per super: even -> DVE, odd -> ACT
    own = [s % 2 for s in range(n_supers)]
    dve_list = [s for s in range(n_supers) if own[s] == 0]
    act_list = [s for s in range(n_supers) if own[s] == 1]
    s2stream = {}
    for j, s in enumerate(dve_list):
        s2stream[s] = (0, j)
    for j, s in enumerate(act_list):
        s2stream[s] = (1, j)
    nb = [(len(dve_list) + STB - 1) // STB, (len(act_list) + STB - 1) // STB]
    # f4 load chunks: small leading chunks so the PE can start early
    fbounds = [0, 2, 4]
    while fbounds[-1] < n_slots:
        fbounds.append(min(n_slots, fbounds[-1] + 4))
    n_f4ch = len(fbounds) - 1
    slot_chunk = {}
    for k in range(n_f4ch):
        for l in range(fbounds[k], fbounds[k + 1]):
            slot_chunk[l] = k
    # rhs load slabs: small leading slab, then even 256-aligned splits
    rs = ((ncol - 1024 + (RSLAB_N - 2)) // (RSLAB_N - 1) + SUP - 1) // SUP * SUP
    rbounds = [0, 1024]
    while rbounds[-1] < ncol:
        rbounds.append(min(ncol, rbounds[-1] + rs))
    n_rslab = len(rbounds) - 1

    # ---- static matmul schedule: (slot|'T', q, a, b, super) ----
    # segment-major: all 4 channel chunks of a super-segment before the
    # next segment, so supers close strictly in column order (a slot may
    # span many supers; q-major would open block s%4 before closing s-4).
    sched = []
    col = 0
    for l in [*range(n_slots), "T"]:
        hi = ncol if l == "T" else col + quota[l]
        a = col
        while a < hi:
            s = a // SUP
            b = min(hi, (s + 1) * SUP)
            for q in range(4):
                sched.append((l, q, a, b, s))
            a = b
        col = hi
    first_touch = {}
    last_touch = {}
    for i, ins in enumerate(sched):
        s = ins[4]
        first_touch.setdefault(s, i)
        last_touch[s] = i
    assert set(first_touch) == set(range(n_supers)), "super coverage hole"
    lt = [last_touch[s] for s in range(n_supers)]
    assert lt == sorted(lt), "non-monotone super retirement"
    inc_at = {i: s for s, i in last_touch.items()}
    wait_at = {i: s for s, i in first_touch.items()}

    nc = bacc.Bacc("TRN2", debug=False)
    f16, f32 = mybir.dt.float16, mybir.dt.float32

    f4_d = nc.dram_tensor("f4", [128, n_slots * C], f16, kind="ExternalInput")
    tl_d = nc.dram_tensor("tl", [128, C], f16, kind="ExternalInput")
    rhs_d = nc.dram_tensor("rhs", [128, ncol], f16, kind="ExternalInput")
    out_d = [
        nc.dram_tensor(nm, [nbk, 128, STB * 1024], f16, kind="ExternalOutput")
        for nm, nbk in (("out_dve", nb[0]), ("out_act", nb[1]))
    ]

    f4 = nc.alloc_sbuf_tensor("f4_sb", [128, n_slots * C], f16)
    tl = nc.alloc_sbuf_tensor("tl_sb", [128, C], f16)
    rhs = nc.alloc_sbuf_tensor("rhs_sb", [128, ncol], f16)
    st = [
        nc.alloc_sbuf_tensor(f"st{e}", [128, NRING * 1024], f16) for e in range(2)
    ]
    ps = nc.alloc_psum_tensor("ps", [128, 4096], f32)

    f_sems = [nc.alloc_semaphore(f"f_sem{i}") for i in range(n_f4ch)]
    t_sem = nc.alloc_semaphore("t_sem")
    r_sems = [nc.alloc_semaphore(f"r_sem{i}") for i in range(n_rslab)]
    pe_sem = nc.alloc_semaphore("pe_sem")
    cast_sems = [nc.alloc_semaphore(f"cast_sem{e}") for e in range(2)]
    # one sem per store-ring position: same-position stores are serialized
    # by the ring-reuse cast gating, so thresholds are unambiguous even
    # with out-of-order DMA completions across positions.
    NPOS = NRING // STB
    st_sems = [
        [nc.alloc_semaphore(f"st_sem{e}_{p}") for p in range(NPOS)]
        for e in range(2)
    ]

    def cast_wait_for(engine, s):
        """Wait until super s is cast (psum block reusable)."""
        e, j = s2stream[s]
        engine.wait_ge(cast_sems[e], j + 1)

    def emit_cast(engine, e, s, j, copy):
        engine.wait_ge(pe_sem, s + 1)
        if j >= NRING:
            k_need = j // STB - NPOS          # store batch freeing this buf
            engine.wait_ge(st_sems[e][k_need % NPOS], 16 * (k_need // NPOS + 1))
        dst = st[e][:, (j % NRING) * 1024 : (j % NRING) * 1024 + 1024]
        src = ps[:, (s % 4) * 1024 : (s % 4) * 1024 + 1024]
        copy(dst, src).then_inc(cast_sems[e], 1)

    with nc.Block() as block:

        @block.scalar
        def _(scalar):
            # only the two PE-critical first loads, then drain odd supers;
            # the bulk of the loads is issued by the idle gpsimd engine.
            scalar.dma_start(
                f4[:, : fbounds[1] * C], f4_d[:, : fbounds[1] * C]
            ).then_inc(f_sems[0], 16)
            scalar.dma_start(
                rhs[:, : rbounds[1]], rhs_d[:, : rbounds[1]]
            ).then_inc(r_sems[0], 16)
            for j, s in enumerate(act_list):
                emit_cast(scalar, 1, s, j, scalar.copy)

        @block.gpsimd
        def _(gp):
            nch = max(n_f4ch, n_rslab)
            for k in range(1, nch):
                if k < n_f4ch:
                    c0, c1 = fbounds[k] * C, fbounds[k + 1] * C
                    gp.dma_start(f4[:, c0:c1], f4_d[:, c0:c1]).then_inc(
                        f_sems[k], 16
                    )
                if k < n_rslab:
                    a, b = rbounds[k], rbounds[k + 1]
                    gp.dma_start(rhs[:, a:b], rhs_d[:, a:b]).then_inc(
                        r_sems[k], 16
                    )
                if k == 2:
                    gp.dma_start(tl[:, :], tl_d[:, :]).then_inc(t_sem, 16)

        @block.vector
        def _(vector):
            for j, s in enumerate(dve_list):
                emit_cast(vector, 0, s, j, vector.tensor_copy)

        @block.tensor
        def _(tensor):
            seen_slot = set()
            rmax = [0]
            for i, (l, q, a, b, s) in enumerate(sched):
                if l not in seen_slot:
                    seen_slot.add(l)
                    if l == "T":
                        tensor.wait_ge(t_sem, 16)
                    else:
                        tensor.wait_ge(f_sems[slot_chunk[l]], 16)
                while rmax[0] < n_rslab and rbounds[rmax[0]] < b:
                    tensor.wait_ge(r_sems[rmax[0]], 16)
                    rmax[0] += 1
                if i in wait_at and wait_at[i] >= 4:
                    cast_wait_for(tensor, wait_at[i] - 4)
                if l == "T":
                    lhsT = tl[:, 128 * q : 128 * (q + 1)]
                else:
                    lhsT = f4[:, l * C + 128 * q : l * C + 128 * (q + 1)]
                off = (s % 4) * 1024 + q * SUP + (a - s * SUP)
                mm = tensor.matmul(
                    ps[:, off : off + (b - a)],
                    lhsT,
                    rhs[:, a:b],
                    start=True,
                    stop=True,
                    skip_group_check=True,
                )
                if i in inc_at:
                    mm.then_inc(pe_sem, 1)

        @block.sync
        def _(sync):
            # all stores, both streams, ordered by global super time
            batches = []
            for e in range(2):
                n_e = len((dve_list, act_list)[e])
                for k in range(nb[e]):
                    last_s = ((dve_list, act_list)[e])[min(STB * (k + 1) - 1, n_e - 1)]
                    batches.append((last_s, e, k, min(STB * (k + 1), n_e)))
            batches.sort()
            for _, e, k, cth in batches:
                sync.wait_ge(cast_sems[e], cth)
                r0 = (STB * k % NRING) * 1024
                sync.dma_start(
                    out_d[e][k], st[e][:, r0 : r0 + STB * 1024]
                ).then_inc(st_sems[e][k % NPOS], 16)
            for e in range(2):
                for p in range(NPOS):
                    cnt = (nb[e] - p + NPOS - 1) // NPOS if nb[e] > p else 0
                    if cnt:
                        sync.wait_ge(st_sems[e][p], 16 * cnt)

    nc.compile()
    return nc


def _get_nc(plan):
    key = hashlib.sha256(
        np.asarray(plan["quota"], np.int64).tobytes()
        + np.int64(plan["quota_tail"]).tobytes()
    ).hexdigest()
    if key not in _NC_CACHE:
        _NC_CACHE[key] = _build_nc(plan)
    return _NC_CACHE[key]


def _host_prep(feats, boxes, img_height, img_width):
    """Per-sample base row (y0*255 + x0, clamped) and 4 slot weights
    (tl, tr, bl, br with validity and clamp-aggregation folded in),
    mirroring the reference math in f32."""
    B = boxes.shape[0]
    f32 = np.float32
    xc, yc, w, h = (boxes[:, k].astype(f32) for k in range(4))
    tx = np.linspace(-1.0, 1.0, WW, dtype=f32)
    ty = np.linspace(-1.0, 1.0, HH, dtype=f32)
    inv_w = f32(1.0) / f32(img_width - 1)
    inv_h = f32(1.0) / f32(img_height - 1)
    gx = (f32(2.0) * xc[:, None] - f32(img_width - 1)) * inv_w \
        + (w * inv_w)[:, None] * tx[None, :]
    gy = (f32(2.0) * yc[:, None] - f32(img_height - 1)) * inv_h \
        + (h * inv_h)[:, None] * ty[None, :]
    px = (gx + f32(1.0)) * f32(0.5) * f32(Wf - 1)   # (B, WW)
    py = (gy + f32(1.0)) * f32(0.5) * f32(Hf - 1)   # (B, HH)

    x0 = np.floor(px)
    y0 = np.floor(py)
    fx, fy = px - x0, py - y0
    x0i, y0i = x0.astype(np.int64), y0.astype(np.int64)
    x1i, y1i = x0i + 1, y0i + 1
    vx0 = ((x0i >= 0) & (x0i <= Wf - 1)).astype(f32)
    vx1 = ((x1i >= 0) & (x1i <= Wf - 1)).astype(f32)
    vy0 = ((y0i >= 0) & (y0i <= Hf - 1)).astype(f32)
    vy1 = ((y1i >= 0) & (y1i <= Hf - 1)).astype(f32)
    x0c = np.clip(x0i, 0, Wf - 1).astype(np.int32)
    x1c = np.clip(x1i, 0, Wf - 1).astype(np.int32)
    y0c = np.clip(y0i, 0, Hf - 1).astype(np.int32)
    y1c = np.clip(y1i, 0, Hf - 1).astype(np.int32)

    def by(a):
        return np.broadcast_to(a[:, :, None], (B, HH, WW))

    def bx(a):
        return np.broadcast_to(a[:, None, :], (B, HH, WW))

    base_y = np.clip(y0i, 0, NPY - 1)                 # (B, HH)
    base_x = np.clip(x0i, 0, NPX - 1)                 # (B, WW)
    rows = (by(base_y) * NPX + bx(base_x)).reshape(-1).astype(np.int32)

    wx0, wx1 = f32(1.0) - fx, fx
    wy0, wy1 = f32(1.0) - fy, fy
    wk = np.stack(
        [
            by(wy0 * vy0) * bx(wx0 * vx0),
            by(wy0 * vy0) * bx(wx1 * vx1),
            by(wy1 * vy1) * bx(wx0 * vx0),
            by(wy1 * vy1) * bx(wx1 * vx1),
        ],
        axis=-1,
    ).reshape(B * HH * WW, 4).astype(f32)
    dy = np.stack(
        [by(y0c - base_y), by(y0c - base_y), by(y1c - base_y), by(y1c - base_y)],
        axis=-1,
    ).reshape(B * HH * WW, 4)
    dx = np.stack(
        [bx(x0c - base_x), bx(x1c - base_x), bx(x0c - base_x), bx(x1c - base_x)],
        axis=-1,
    ).reshape(B * HH * WW, 4)
    slots = np.clip(dy, 0, 1) * 2 + np.clip(dx, 0, 1)
    wts = np.zeros((B * HH * WW, 4), f32)
    np.add.at(wts, (np.arange(B * HH * WW)[:, None], slots), wk)
    return rows, wts


def _prepare(feats, boxes, img_height, img_width):
    rows, wts = _host_prep(feats, boxes, img_height, img_width)
    n = rows.shape[0]
    y0 = (rows // NPX).astype(np.int64)          # 0..62
    x0 = (rows % NPX).astype(np.int64)           # 0..254
    blk = np.minimum(x0 // XBW, NBLK)            # 0..4 (4 = tail)
    is_tail = blk == NBLK

    # per-cell sample id lists (stable order)
    cell_of = y0 * (NBLK + 1) + blk
    order = np.argsort(cell_of, kind="stable")
    co = cell_of[order]
    starts = np.r_[0, np.flatnonzero(co[1:] != co[:-1]) + 1]
    uniq = co[starts]
    lens = np.diff(np.r_[starts, n])
    cell_ids = {int(u): order[s : s + L] for u, s, L in zip(uniq, starts, lens)}

    # split non-tail cells into pieces of <= SPLIT_MAX columns
    pieces = []                                   # (size, v, b, off)
    for v in range(NPY):
        for b in range(NBLK):
            ids = cell_ids.get(v * (NBLK + 1) + b)
            if ids is None:
                continue
            cnum = len(ids)
            k = -(-cnum // SPLIT_MAX)
            base, rem = cnum // k, cnum % k
            off = 0
            for j in range(k):
                sz = base + (1 if j < rem else 0)
                pieces.append((sz, v, b, off))
                off += sz
    pieces.sort(reverse=True)

    # LPT deal to cores; per-core lists stay size-sorted by re-sorting
    heap = [(0, 0, m) for m in range(N_CORES)]
    heapq.heapify(heap)
    percore = [[] for _ in range(N_CORES)]
    for p in pieces:
        tot, ns, m = heapq.heappop(heap)
        percore[m].append(p)
        heapq.heappush(heap, (tot + p[0], ns + 1, m))
    for m in range(N_CORES):
        percore[m].sort(reverse=True)
    n_slots = max(len(p) for p in percore)
    quota = np.zeros(n_slots, np.int64)
    for m in range(N_CORES):
        for l, p in enumerate(percore[m]):
            quota[l] = max(quota[l], p[0])

    # tail cells: snake-deal by size (<= 16 per core)
    tcells = sorted(
        (
            (len(cell_ids[v * (NBLK + 1) + NBLK]), v)
            for v in range(NPY)
            if v * (NBLK + 1) + NBLK in cell_ids
        ),
        reverse=True,
    )
    tcore = [[] for _ in range(N_CORES)]
    ttot = np.zeros(N_CORES, np.int64)
    for i, c in enumerate(tcells):
        r, m = divmod(i, N_CORES)
        m = m if r % 2 == 0 else N_CORES - 1 - m
        tcore[m].append(c)
        ttot[m] += c[0]
    assert max(len(t) for t in tcore) <= 16, "tail tile overflow"
    qt = int(ttot.max())
    ncol = int(quota.sum()) + qt
    qt += (-ncol) % SUP
    ncol += (-ncol) % SUP
    plan = {"quota": quota, "quota_tail": qt, "ncol": ncol}

    slot_start = np.zeros(n_slots + 1, np.int64)
    np.cumsum(quota, out=slot_start[1:])
    tail_start = int(slot_start[n_slots])

    ftab = feats.astype(np.float16)               # (C, Hf, Wf)
    yp_ = np.arange(128) // 64
    xo_ = np.arange(128) % 64

    in_maps, colmaps = [], []
    for m in range(N_CORES):
        f4_dat = np.zeros((128, n_slots * C), np.float16)
        rhs = np.zeros((128, ncol), np.float16)
        colmap = np.full(ncol, -1, np.int64)
        for l, (sz, v, b, off) in enumerate(percore[m]):
            ids = cell_ids[v * (NBLK + 1) + b][off : off + sz]
            # tile: [p = yp*64 + xo, c] = feats[v + yp, 63*b + xo, c]
            f4_dat[:, l * C : (l + 1) * C] = ftab[
                :, v + yp_, XBW * b + xo_
            ].T
            cols = slot_start[l] + np.arange(sz)
            colmap[cols] = ids
            xo = x0[ids] - XBW * b
            w4 = wts[ids]
            rhs[xo, cols] = w4[:, 0]
            rhs[xo + 1, cols] = w4[:, 1]
            rhs[64 + xo, cols] = w4[:, 2]
            rhs[64 + xo + 1, cols] = w4[:, 3]
        # tail tile: [p = u*8 + yp*4 + xoff, c] = feats[v_u + yp, 252 + xoff, c]
        tl_dat = np.zeros((128, C), np.float16)
        tcol = tail_start
        for u, (sz, v) in enumerate(tcore[m]):
            p_ = np.arange(8)
            tl_dat[u * 8 + p_] = ftab[
                :, v + p_ // 4, np.minimum(NBLK * XBW + p_ % 4, Wf - 1)
            ].T
            ids = cell_ids[v * (NBLK + 1) + NBLK]
            cols = tcol + np.arange(sz)
            tcol += sz
            colmap[cols] = ids
            xoff = x0[ids] - NBLK * XBW
            w4 = wts[ids]
            rhs[u * 8 + xoff, cols] = w4[:, 0]
            rhs[u * 8 + xoff + 1, cols] = w4[:, 1]
            rhs[u * 8 + 4 + xoff, cols] = w4[:, 2]
            rhs[u * 8 + 4 + xoff + 1, cols] = w4[:, 3]
        in_maps.append(
            {
                "f4": f4_dat,
                "tl": tl_dat,
                "rhs": np.ascontiguousarray(rhs),
            }
        )
        colmaps.append(colmap)

    return plan, in_maps, colmaps


def kernel(**inputs):
    from concourse.bass_utils import run_bass_kernel_spmd

    feats = np.asarray(inputs["feats"], dtype=np.float32)
    boxes = np.asarray(inputs["boxes"], dtype=np.float32)
    img_height = int(np.asarray(inputs["img_height"]))
    img_width = int(np.asarray(inputs["img_width"]))

    plan, in_maps, colmaps = _prepare(feats, boxes, img_height, img_width)
    nc = _get_nc(plan)
    res = run_bass_kernel_spmd(nc, in_maps, core_ids=list(range(N_CORES)))

    ncol = plan["ncol"]
    n_supers = ncol // SUP
    out_all = np.empty((C, B_TOTAL * HH * WW), np.float32)
    for m, r in enumerate(res.results):
        full = np.empty((C, ncol), np.float32)
        for e, nm in enumerate(("out_dve", "out_act")):
            a = r[nm]                              # (nb, 128, 4*1024) f16
            slist = [s for s in range(n_supers) if s % 2 == e]
            nbk = a.shape[0]
            # a[k, p, j*1024 + q*256 + r] -> [q*128 + p, (k*STB + j)*SUP + r]
            x = (
                a.reshape(nbk, 128, STB, 4, SUP)
                .transpose(3, 1, 0, 2, 4)
                .reshape(C, nbk * STB * SUP)
            )
            for j, s in enumerate(slist):
                full[:, s * SUP : (s + 1) * SUP] = x[
                    :, j * SUP : (j + 1) * SUP
                ].astype(np.float32)
        cm = colmaps[m]
        valid = cm >= 0
        out_all[:, cm[valid]] = full[:, valid]
    out = out_all.T.reshape(B_TOTAL, HH * WW, C).transpose(0, 2, 1)
    return np.ascontiguousarray(out.reshape(B_TOTAL, C, HH, WW)).astype(np.float32)
